# revision 4
# baseline (speedup 1.0000x reference)
"""AdditiveAttention Trainium2 kernel (8 NeuronCores, SPMD, no collectives).

reference:
    q = queries @ Wq               (B,Q,H)
    k = keys @ Wk                  (B,K,H)
    scores[b,q,k] = sum_h wv[h] * tanh(q[b,q,h] + k[b,k,h])
    masked = where(arange(K) < valid_lens[b], scores, 0.0)
    attn = softmax(masked, -1)      # masked cols contribute e^0 = 1
    out = attn @ values             (B,Q,D)

Sharding: core c = (b, q_half) -> computes out[b, qh*128:(qh+1)*128, :].
Each core owns 128 queries x full K of one batch. Purely data-parallel,
no cross-core reduction.

Per-core structure (h=H=128 on partitions for the score stage):
  - kpT[h, k], qpT[h, q] via PE matmuls (f32)
  - per q: ONE ScalarE activation computes tanh(kpT + qpT[:, q]) fused
    (per-partition bias), output bf16 [128, KE]
  - per q: PE matmuls with a "sliding window" stationary operand
    (wv at column q, zeros elsewhere) accumulate that q's scores into
    row q of a PSUM tile -> dense scores[q, k] with wv never reloaded
    per-row layouts
  - mask multiply (masked logits -> 0), exp on ScalarE with accum_out
    giving the softmax denominator for free
  - PE transpose of E -> E_T, then attn@V matmuls; k >= KE tail handled
    by all-ones stationary operand (exp(0) == 1 there)
  - normalize with per-partition 1/Z and DMA out

KE = ceil(max(valid_lens)/128)*128 <= K: columns >= KE are masked in every
batch, so tanh/exp work shrinks to KE columns.
"""

import sys

sys.path.insert(0, "/opt/trn_rl_repo")

from contextlib import ExitStack

import numpy as np
import ml_dtypes

import concourse.bass as bass
import concourse.mybir as mybir
import concourse.tile as tile
from concourse import bacc
from concourse.bass_utils import run_bass_kernel_spmd
from concourse.masks import make_identity

B, Q, K, D, H = 4, 256, 1024, 512, 128
QS = Q // 2  # queries per core
N_CORES = 8
F32 = mybir.dt.float32
BF16 = mybir.dt.bfloat16
BF16_NP = np.dtype(ml_dtypes.bfloat16)


def build_graph(KE: int) -> bass.Bass:
    assert KE % 128 == 0 and 128 <= KE <= K
    DC = D // 128  # contraction chunks for the projections
    # n-chunks (<=512) of the score/exp free axis
    k_chunks = [(s, min(512, KE - s)) for s in range(0, KE, 512)]
    KC128 = KE // 128
    VC = K // 128

    nc = bacc.Bacc("TRN2", target_bir_lowering=False, debug=False)

    qT_d = nc.declare_dram_parameter("qT", [D, QS], F32, isOutput=False)
    kT_d = nc.declare_dram_parameter("kT", [D, KE], F32, isOutput=False)
    v_d = nc.declare_dram_parameter("v", [K, D], F32, isOutput=False)
    wq_d = nc.declare_dram_parameter("wq", [D, H], F32, isOutput=False)
    wk_d = nc.declare_dram_parameter("wk", [D, H], F32, isOutput=False)
    # [128, 512] bf16: col 126 = wv (even-q window), col 256+127 = wv (odd-q)
    wvwin_d = nc.declare_dram_parameter("wvwin", [H, 512], BF16, isOutput=False)
    mask_d = nc.declare_dram_parameter("mask", [QS, KE], F32, isOutput=False)
    out_d = nc.declare_dram_parameter("out", [QS, D], F32, isOutput=True)

    with tile.TileContext(nc) as tc, ExitStack() as ctx:
        const = ctx.enter_context(tc.tile_pool(name="const", bufs=1))
        work = ctx.enter_context(tc.tile_pool(name="work", bufs=1))
        tq_pool = ctx.enter_context(tc.tile_pool(name="tq", bufs=3))
        pp = ctx.enter_context(tc.tile_pool(name="pp", bufs=1, space="PSUM"))
        scp = ctx.enter_context(tc.tile_pool(name="scp", bufs=1, space="PSUM"))
        tpp = ctx.enter_context(tc.tile_pool(name="tpp", bufs=2, space="PSUM"))
        pop = ctx.enter_context(tc.tile_pool(name="pop", bufs=1, space="PSUM"))

        # ---- load inputs ----
        qT_sb = [const.tile([128, QS], F32, tag=f"qT{i}", name=f"qT{i}") for i in range(DC)]
        kT_sb = [const.tile([128, KE], F32, tag=f"kT{i}", name=f"kT{i}") for i in range(DC)]
        wq_sb = [const.tile([128, H], F32, tag=f"wq{i}", name=f"wq{i}") for i in range(DC)]
        wk_sb = [const.tile([128, H], F32, tag=f"wk{i}", name=f"wk{i}") for i in range(DC)]
        v_sb = [const.tile([128, D], F32, tag=f"v{i}", name=f"v{i}") for i in range(VC)]
        wvwin_sb = const.tile([H, 512], BF16, tag="wvwin")
        mask_sb = const.tile([QS, KE], F32, tag="mask")
        for i in range(DC):
            sl = slice(i * 128, (i + 1) * 128)
            nc.sync.dma_start(qT_sb[i][:], qT_d[sl, :])
            nc.sync.dma_start(kT_sb[i][:], kT_d[sl, :])
            nc.sync.dma_start(wq_sb[i][:], wq_d[sl, :])
            nc.sync.dma_start(wk_sb[i][:], wk_d[sl, :])
        for i in range(VC):
            nc.sync.dma_start(v_sb[i][:], v_d[i * 128 : (i + 1) * 128, :])
        nc.sync.dma_start(wvwin_sb[:], wvwin_d[:, :])
        nc.sync.dma_start(mask_sb[:], mask_d[:, :])

        ident = const.tile([128, 128], F32, tag="ident")
        make_identity(nc, ident[:])
        ones_sb = const.tile([128, 128], F32, tag="ones")
        nc.gpsimd.memset(ones_sb[:], 1.0)

        # ---- projections: qpT[h, q], kpT[h, k] ----
        qp_ps = pp.tile([H, QS], F32, tag="qp_ps")
        for i in range(DC):
            nc.tensor.matmul(
                qp_ps[:], wq_sb[i][:], qT_sb[i][:], start=(i == 0), stop=(i == DC - 1)
            )
        kp_ps = pp.tile([H, KE], F32, tag="kp_ps")
        for s, w in k_chunks:
            for i in range(DC):
                nc.tensor.matmul(
                    kp_ps[:, s : s + w],
                    wk_sb[i][:],
                    kT_sb[i][:, s : s + w],
                    start=(i == 0),
                    stop=(i == DC - 1),
                )
        qp_sb = work.tile([H, QS], F32, tag="qp_sb")
        nc.vector.tensor_copy(qp_sb[:], qp_ps[:])
        kp_sb = work.tile([H, KE], F32, tag="kp_sb")
        nc.vector.tensor_copy(kp_sb[:], kp_ps[:])

        # ---- per-q fused tanh + wv-projection scatter ----
        sc_ps = scp.tile([QS, KE], F32, tag="sc_ps")
        for q in range(QS):
            tq = tq_pool.tile([H, KE], BF16, tag="tq")
            nc.scalar.activation(
                tq[:],
                kp_sb[:],
                mybir.ActivationFunctionType.Tanh,
                bias=qp_sb[:, q : q + 1],
            )
            if q % 2 == 0:
                off = 126 - q
            else:
                off = 256 + 127 - q
            win = wvwin_sb[:, off : off + 128]
            for s, w in k_chunks:
                nc.tensor.matmul(
                    sc_ps[:, s : s + w],
                    win,
                    tq[:, s : s + w],
                    start=(q == 0),
                    stop=(q == QS - 1),
                )

        # ---- mask, exp (+ row-sum Z) ----
        msk_sb = work.tile([QS, KE], F32, tag="msk_sb")
        nc.vector.tensor_mul(msk_sb[:], sc_ps[:], mask_sb[:])
        e_sb = work.tile([QS, KE], F32, tag="e_sb")
        z_sb = work.tile([QS, 1], F32, tag="z_sb")
        nc.scalar.activation(
            e_sb[:],
            msk_sb[:],
            mybir.ActivationFunctionType.Exp,
            accum_out=z_sb[:],
        )

        # ---- transpose E -> E_T ----
        et_sb = work.tile([128, KC128 * QS], F32, tag="et_sb")
        for c in range(KC128):
            tp = tpp.tile([128, 128], F32, tag="tp")
            nc.tensor.transpose(tp[:], e_sb[:, c * 128 : (c + 1) * 128], ident[:])
            nc.vector.tensor_copy(et_sb[:, c * QS : (c + 1) * QS], tp[:])

        # ---- attn @ V  (tail chunks use all-ones: exp(0) = 1) ----
        po = pop.tile([QS, D], F32, tag="po")
        for c in range(VC):
            lhsT = et_sb[:, c * QS : (c + 1) * QS] if c < KC128 else ones_sb[:]
            nc.tensor.matmul(
                po[:],
                lhsT,
                v_sb[c][:],
                start=(c == 0),
                stop=(c == VC - 1),
            )

        # ---- normalize and store ----
        z2 = work.tile([QS, 1], F32, tag="z2")
        nc.vector.tensor_scalar_add(z2[:], z_sb[:], float(K - KE))
        rz = work.tile([QS, 1], F32, tag="rz")
        nc.vector.reciprocal(rz[:], z2[:])
        out_sb = work.tile([QS, D], F32, tag="out_sb")
        nc.vector.tensor_scalar_mul(out_sb[:], po[:], rz[:])
        nc.sync.dma_start(out_d[:, :], out_sb[:])

    nc.compile()
    return nc


_GRAPH_CACHE: dict[int, bass.Bass] = {}
_LAST_RESULTS = None


def _get_graph(KE: int) -> bass.Bass:
    if KE not in _GRAPH_CACHE:
        _GRAPH_CACHE[KE] = build_graph(KE)
    return _GRAPH_CACHE[KE]


def make_in_maps(queries, keys, values, Wq, Wk, wv, valid_lens, KE):
    wvwin = np.zeros((H, 512), BF16_NP)
    wvwin[:, 126] = wv.astype(BF16_NP)
    wvwin[:, 256 + 127] = wv.astype(BF16_NP)
    col = np.arange(KE)
    in_maps = []
    for c in range(N_CORES):
        b, qh = divmod(c, 2)
        mask_row = (col < int(valid_lens[b])).astype(np.float32)
        in_maps.append(
            {
                "qT": np.ascontiguousarray(
                    queries[b, qh * QS : (qh + 1) * QS, :].T
                ),
                "kT": np.ascontiguousarray(keys[b, :KE, :].T),
                "v": np.ascontiguousarray(values[b]),
                "wq": np.ascontiguousarray(Wq),
                "wk": np.ascontiguousarray(Wk),
                "wvwin": wvwin,
                "mask": np.ascontiguousarray(
                    np.broadcast_to(mask_row, (QS, KE))
                ),
            }
        )
    return in_maps


def kernel(queries, keys, values, Wq, Wk, wv, valid_lens, **run_kwargs):
    queries = np.asarray(queries, np.float32)
    keys = np.asarray(keys, np.float32)
    values = np.asarray(values, np.float32)
    Wq = np.asarray(Wq, np.float32)
    Wk = np.asarray(Wk, np.float32)
    wv = np.asarray(wv, np.float32)
    valid_lens = np.asarray(valid_lens, np.int32)

    KE = int(-(-int(valid_lens.max()) // 128) * 128)
    KE = max(128, min(K, KE))

    nc = _get_graph(KE)
    in_maps = make_in_maps(queries, keys, values, Wq, Wk, wv, valid_lens, KE)
    res = run_bass_kernel_spmd(
        nc, in_maps, core_ids=list(range(N_CORES)), **run_kwargs
    )
    global _LAST_RESULTS
    _LAST_RESULTS = res
    out = np.empty((B, Q, D), np.float32)
    for c in range(N_CORES):
        b, qh = divmod(c, 2)
        out[b, qh * QS : (qh + 1) * QS, :] = res.results[c]["out"]
    return out
